# revision 23
# baseline (speedup 1.0000x reference)
"""Trainium2 Bass kernel for nn_CustomLossMinMax.

Computes, over full inputs pos_outputs [N,L], neg_outputs [M,L], p [N,L]
(N=M=8192, L=2048, f32):

    wpos[i]   = sum_l pos[i,l] * p[i,l]
    negmax[j] = max_l neg[j,l]
    out       = sum_ij relu(1 - wpos[i] + negmax[j]) / (N*M)

Sharding (8 cores): rows of pos/p and rows of neg are split 1024/core.
Each core computes its wpos shard and negmax shard, AllGathers the tiny
(1 + negmax) vector in bf16, replicates it across partitions, then
accumulates the pairwise hinge for its own 1024 i-rows against all 8192
j's. Per-core partial sums are summed on the host and scaled by 1/(N*M).

Schedule per core (24 MiB of input DMA is the roofline):
  - A dummy 16-element AllGather is triggered at t~0 so the one-time
    device barrier (~22us) and CC-proxy cold start are absorbed while
    the neg tiles stream in; the real AllGathers then run warm.
  - neg tiles stream first on both HWDGE rings (2 outstanding per ring
    so arrivals stay sequential and the row-max reduces pipeline);
    pos/p tiles follow in the same ring FIFOs, so neg has strict
    priority without cross-ring dep edges.
  - Collective bounce-in DMAs sit at SP-ring FIFO positions where their
    dependency wait cannot stall anything that matters; the gathered-row
    fetches are gpsimd SWDGE DMAs interleaved with the AG chain
    (AG1, fetch_a, AG2, fetch_b) so each chunk is fetched immediately.
  - Partition broadcast per j-chunk: TensorE ones-matmuls fill PSUM,
    one DVE copy bridges PSUM->SBUF (bf16), then both ACT and DVE
    stream the pairwise from SBUF.
  - Pairwise hinge: ACT uses activation(Relu, bias=-wpos, accum) at 1x;
    DVE uses tensor_scalar(add -wpos, max 0, accum) which runs in the
    16-bit 4x perf mode - so DVE takes the late i-tiles (short tail)
    and ACT the early ones.

All i/j orderings inside the kernel are permutations of the reference
ordering; the final scalar sum is permutation-invariant. The bf16
transport of (1+negmax) perturbs the result by ~5e-6 relative.
"""
import sys
import numpy as np

for _p in ("/opt/trn_rl_repo", "/root/.axon_site/_ro/trn_rl_repo"):
    if _p not in sys.path:
        sys.path.insert(0, _p)

from concourse import bacc, mybir, tile  # noqa: E402
from concourse import bass_utils  # noqa: E402
from concourse.tile_rust import add_dep_helper  # noqa: E402

N_CORES = 8
N, M, L = 8192, 8192, 2048
ROWS = N // N_CORES          # 1024 rows per core for pos/p and neg
T = ROWS // 128              # 8 row-tiles of 128 partitions per core
F32 = mybir.dt.float32
BF16 = mybir.dt.bfloat16

# ---- build-time schedule config ------------------------------------
CFG = dict(
    dummy_ag=True,            # tiny AllGather at t~0 absorbs the barrier
    chunks=2,                 # j-chunks: 1 = single AG, 2 = split AG
    act_tiles=((0, 1, 2, 3, 4), (0, 1, 2, 3)),  # pairwise tiles on ACT
    # emission order IS program order for the dependency tracker, so
    # each engine gets its own copy of chunk A's broadcast row: ACT
    # self-copies (post-loop, just before its units), DVE copies at
    # wpos-slot copy_slots[0]; chunk B has one DVE copy at slot [1]
    # that both engines' B-units read.
    copy_slots=(3, 6),
    dve_unit_gate=(4, 6),
)

_cache = {}


def _build():
    chunks = CFG["chunks"]
    act_tiles = [set(x) for x in CFG["act_tiles"]]
    dve_tiles = [[t for t in range(T) if t not in act_tiles[h]]
                 for h in range(chunks)]
    TH = T // chunks         # negmax columns per AG chunk
    HM = M // chunks         # j-columns per chunk
    QN = HM // 4096          # PSUM-sized sub-blocks per chunk

    nc = bacc.Bacc("TRN2", target_bir_lowering=False, debug=False,
                   enable_asserts=True, num_devices=N_CORES)
    pos = nc.dram_tensor("pos", [ROWS, L], F32, kind="ExternalInput").ap()
    p = nc.dram_tensor("p", [ROWS, L], F32, kind="ExternalInput").ap()
    neg = nc.dram_tensor("neg", [ROWS, L], F32, kind="ExternalInput").ap()
    out = nc.dram_tensor("partial", [128, 2 * T], F32,
                         kind="ExternalOutput").ap()

    pos_t = pos.rearrange("(t p) l -> t p l", p=128)
    p_t = p.rearrange("(t p) l -> t p l", p=128)
    neg_t = neg.rearrange("(t p) l -> t p l", p=128)

    with tile.TileContext(nc) as tc:
        with tc.tile_pool(name="negp", bufs=5) as neg_pool, \
             tc.tile_pool(name="posp", bufs=5) as pos_pool, \
             tc.tile_pool(name="scrp", bufs=1) as scr_pool, \
             tc.tile_pool(name="big", bufs=1) as big_pool, \
             tc.tile_pool(name="small", bufs=1) as small_pool, \
             tc.tile_pool(name="psum", bufs=1, space="PSUM") as psum_pool, \
             tc.tile_pool(name="dram", bufs=1, space="DRAM") as dpool:

            # all-ones row for the TensorE partition-broadcast
            ones_bf = small_pool.tile([1, 128], BF16)
            nc.vector.memset(ones_bf[:], 1.0)
            acc = small_pool.tile([128, 2 * T], F32)
            nc.vector.memset(acc[:], 0.0)
            # dense bf16 zeros: 2nd operand of the DVE pairwise STT
            zeros_bf = big_pool.tile([128, M // CFG["chunks"]], BF16)
            nc.vector.memset(zeros_bf[:], 0.0)

            # ---- dummy collective: absorb barrier + proxy cold start
            # (very first gpsimd instruction, so its trigger fires as
            # early as the engine start event allows)
            if CFG["dummy_ag"]:
                dummy_sb = small_pool.tile([1, 16], BF16)
                nc.vector.memset(dummy_sb[:], 0.0)
                ccd_in = dpool.tile([1, 16], BF16)
                ccd_out = dpool.tile([N_CORES, 16], BF16)
                nc.sync.dma_start(ccd_in[:], dummy_sb[:])
                nc.gpsimd.collective_compute(
                    "AllGather", mybir.AluOpType.bypass,
                    ins=[ccd_in[:].opt()], outs=[ccd_out[:].opt()],
                    replica_groups=[list(range(N_CORES))])

            # warm up the gpsimd SWDGE DMA path (Q7 ucode IRAM load)
            # behind the dummy trigger, ahead of the gathered-row fetches
            warm = small_pool.tile([1, 16], F32)
            nc.gpsimd.dma_start(warm[:], neg[0:1, 0:16])

            # ---- phase 1: neg streams first; row-max per tile ------
            negmax_sb = small_pool.tile([128, T], F32)
            cc_ins, cc_outs, nm1s = [], [], []
            neg_dmas = []
            for t in range(T):
                ntile = neg_pool.tile([128, L], F32, tag="neg")
                ring = nc.sync if t % 2 == 0 else nc.scalar
                d = ring.dma_start(ntile[:], neg_t[t])
                if t >= 4:
                    # 2 outstanding per ring: arrivals stay sequential
                    add_dep_helper(d.ins, neg_dmas[t - 4], sync=True,
                                   reason="stagger neg arrivals")
                neg_dmas.append(d.ins)
                nc.vector.tensor_reduce(negmax_sb[:, t:t + 1], ntile[:],
                                        axis=mybir.AxisListType.X,
                                        op=mybir.AluOpType.max)
                if (t + 1) % TH == 0:
                    # (1 + negmax chunk) in bf16, ready for bounce-out
                    h = (t + 1) // TH - 1
                    nm1 = small_pool.tile([128, TH], BF16,
                                          name=f"nm1_{h}")
                    nc.vector.tensor_scalar_add(nm1[:],
                                                negmax_sb[:, h * TH:
                                                          (h + 1) * TH], 1.0)
                    nm1s.append(nm1)
                    cc_ins.append(dpool.tile([128, TH], BF16,
                                             name=f"ccin{h}"))
                    cc_outs.append(dpool.tile([128 * N_CORES, TH], BF16,
                                              name=f"ccout{h}"))

            # bounce-in DMAs: SP-ring FIFO slots right after the neg
            # tiles (their tsa dependency resolves before the ring
            # reaches them, and pos issues behind them are not urgent)
            for h in range(chunks):
                nc.sync.dma_start(cc_ins[h][:], nm1s[h][:])

            # ---- phase 2: AG chain + fetches on gpsimd -------------
            negrows = []
            for h in range(chunks):
                nc.gpsimd.collective_compute(
                    "AllGather", mybir.AluOpType.bypass,
                    ins=[cc_ins[h][:].opt()], outs=[cc_outs[h][:].opt()],
                    replica_groups=[list(range(N_CORES))])
                negrow = small_pool.tile([1, HM], BF16, name=f"negrow{h}")
                nc.gpsimd.dma_start(
                    negrow[:],
                    cc_outs[h][:].rearrange("a b -> (a b)")
                    .rearrange("(a b) -> a b", a=1))
                negrows.append(negrow)

            # ---- phase 3: PE broadcast into PSUM -------------------
            assert chunks == 2 and QN == 1
            psums = []
            for h in range(chunks):
                psum_bc = psum_pool.tile([128, HM], F32, tag="psum_bc")
                for k in range(HM // 512):
                    nc.tensor.matmul(
                        psum_bc[:, k * 512:(k + 1) * 512],
                        ones_bf[:], negrows[h][:, k * 512:(k + 1) * 512],
                        start=True, stop=True)
                psums.append(psum_bc)
            # SBUF copies of the broadcast rows: one per engine for
            # chunk A (so each engine's reader is emitted after its own
            # writer), one shared (DVE-written) for chunk B
            bc0_act = big_pool.tile([128, HM], BF16, tag="bc0_act")
            bc0_dve = big_pool.tile([128, HM], BF16, tag="bc0_dve")
            bc1 = big_pool.tile([128, HM], BF16, tag="bc1")
            dve_srcs = (bc0_dve, bc1)
            act_srcs = (bc0_act, bc1)

            # ---- phase 4: pos/p stream -> wpos; pairwise hinge -----
            a_sb = small_pool.tile([128, T], F32)
            wscr = scr_pool.tile([128, L], BF16, tag="wpos_scr")
            act_scr = big_pool.tile([128, HM], BF16, tag="act_scr")
            dve_scr = big_pool.tile([128, HM], BF16, tag="dve_scr")

            pos_last = {}
            emitted = set()
            state = {"copies": 0}

            def emit_dve_units(tmax):
                # all DVE pairwise units whose wpos tile is ready; the
                # per-chunk gate keeps a unit after its chunk's DVE
                # broadcast copy in emission (= stream) order
                for h in range(chunks):
                    if tmax < CFG["dve_unit_gate"][h]:
                        continue
                    if state["copies"] <= h:
                        continue
                    for tt in dve_tiles[h]:
                        if tt <= tmax and (h, tt) not in emitted:
                            u = h * T + tt
                            nc.vector.scalar_tensor_tensor(
                                out=dve_scr[:], in0=dve_srcs[h][:],
                                scalar=a_sb[:, tt:tt + 1], in1=zeros_bf[:],
                                op0=mybir.AluOpType.add,
                                op1=mybir.AluOpType.max,
                                accum_out=acc[:, u:u + 1])
                            emitted.add((h, tt))

            for t in range(T):
                ptile = pos_pool.tile([128, L], F32, tag="pos")
                wtile = pos_pool.tile([128, L], F32, tag="p")
                d0 = nc.sync.dma_start(ptile[:], pos_t[t])
                d1 = nc.scalar.dma_start(wtile[:], p_t[t])
                if t >= 2:
                    add_dep_helper(d0.ins, pos_last[t - 2][0], sync=True,
                                   reason="stagger pos arrivals")
                    add_dep_helper(d1.ins, pos_last[t - 2][1], sync=True,
                                   reason="stagger p arrivals")
                pos_last[t] = (d0.ins, d1.ins)
                # fused a = -wpos row-sum on DVE
                nc.vector.scalar_tensor_tensor(
                    out=wscr[:], in0=ptile[:], scalar=-1.0, in1=wtile[:],
                    op0=mybir.AluOpType.mult, op1=mybir.AluOpType.mult,
                    accum_out=a_sb[:, t:t + 1])
                while (state["copies"] < chunks
                       and t >= CFG["copy_slots"][state["copies"]]):
                    h = state["copies"]
                    nc.vector.tensor_copy(dve_srcs[h][:], psums[h][:])
                    state["copies"] += 1
                emit_dve_units(t)

            # ---- ACT pairwise units (in-order on the ACT stream) ---
            # chunk A's own PSUM->SBUF bridge on ACT: its first unit
            # starts as soon as the AllGather lands
            nc.scalar.copy(bc0_act[:], psums[0][:])
            for h in range(chunks):
                for tt in sorted(act_tiles[h]):
                    u = h * T + tt
                    nc.scalar.activation(
                        act_scr[:], act_srcs[h][:],
                        mybir.ActivationFunctionType.Relu,
                        bias=a_sb[:, tt:tt + 1], scale=1.0,
                        accum_out=acc[:, u:u + 1])

            nc.sync.dma_start(out, acc[:])
    nc.compile()
    return nc


def kernel(pos_outputs: np.ndarray, neg_outputs: np.ndarray,
           p: np.ndarray) -> np.ndarray:
    if "nc" not in _cache:
        _cache["nc"] = _build()
    nc = _cache["nc"]

    pos_outputs = np.ascontiguousarray(pos_outputs, dtype=np.float32)
    neg_outputs = np.ascontiguousarray(neg_outputs, dtype=np.float32)
    p = np.ascontiguousarray(p, dtype=np.float32)

    in_maps = []
    for c in range(N_CORES):
        sl = slice(c * ROWS, (c + 1) * ROWS)
        in_maps.append({
            "pos": pos_outputs[sl],
            "p": p[sl],
            "neg": neg_outputs[sl],
        })
    res = bass_utils.run_bass_kernel_spmd(nc, in_maps,
                                          core_ids=list(range(N_CORES)))
    total = 0.0
    for c in range(N_CORES):
        total += res.results[c]["partial"].astype(np.float64).sum()
    return np.asarray(total / (float(N) * float(M)), dtype=np.float32)
